# revision 37
# baseline (speedup 1.0000x reference)
"""VQ codebook (nn_ConceptGraph) Trainium2 kernel.

Data-parallel over batch B=8: one NeuronCore per batch row (8192 tokens each).
Per core:
  x [8192, 512] f32, codebook [32, 512] f32 (replicated).
  S[t, k] = 2*x.c_k - |x_t|^2 - |c_k|^2  (= -squared_dist)
  idx = argmax_k S  (== argmin dist, first-index tie-break via max8/max_index)
  quantized[t] = codebook[idx[t]]
  counts += onehot(idx), maxsum += max_k S (ones-matmul PSUM accumulation)
Host combines: counts = sum over cores; vq_loss = 1.25 * (-sum maxsum) / (B*T*D).

Pipeline per 512-token group (16 groups/core):
  DMA x -> PE transpose (xT) -> ACT copy to SBUF -> PE matmul vs cT
  (dists in [tok, 32] PSUM) -> DVE argmax chain -> PE onehot-matmul gather
  (bf16-split codebook for full-rate PE) -> DVE/ACT copy -> DMA out.
"""

import os
import sys

import numpy as np

sys.path.insert(0, "/opt/trn_rl_repo")

import concourse.bacc as bacc
import concourse.bass as bass
import concourse.mybir as mybir
import concourse.tile as tile
from concourse.bass import IndirectOffsetOnAxis
from concourse.bass_utils import run_bass_kernel_spmd
from concourse.masks import make_identity
from contextlib import ExitStack

F32 = mybir.dt.float32
BF16 = mybir.dt.bfloat16
I32 = mybir.dt.int32
U32 = mybir.dt.uint32
AF = mybir.ActivationFunctionType
OP = mybir.AluOpType
AX = mybir.AxisListType

P = 128          # partitions
D = 512          # model dim
K = 32           # codebook entries
B = 8            # batch (== n cores)
T = 8192         # tokens per core
NCHUNK = D // P  # 4 contraction chunks

COMMITMENT_COST = 0.25

# tensor_tensor_reduce crashes this runtime's devices — never use it.
GATHER_MODE = os.environ.get("VQ_GATHER", "pe")   # "pe" | "dma"
QMM_SPLIT = int(os.environ.get("VQ_QSPLIT", "3"))  # bf16 split terms for q-matmul
X2_ENGINE = os.environ.get("VQ_X2", "act")         # "act" | "pool"


def build_nc(tok: int = T, group: int = 512):
    ng = tok // group        # number of groups
    tpg = group // P         # tiles (of 128 tokens) per group

    nc = bacc.Bacc("TRN2", target_bir_lowering=False, debug=False)
    xs = nc.declare_dram_parameter("xs", [tok, D], F32, isOutput=False)
    cb = nc.declare_dram_parameter("cb", [K, D], F32, isOutput=False)
    q_out = nc.declare_dram_parameter("q", [tok, D], F32, isOutput=True)
    stats_out = nc.declare_dram_parameter("stats", [1, K + tpg], F32,
                                          isOutput=True)

    # DRAM views: token index = g*group + c*128 + p
    xs_v = xs[:].rearrange("(g c p) d -> g p c d", p=P, c=tpg)
    q_v = q_out[:].rearrange("(g c p) d -> g p c d", p=P, c=tpg)

    with ExitStack() as ctx:
        tc = ctx.enter_context(tile.TileContext(nc))
        consts = ctx.enter_context(tc.tile_pool(name="consts", bufs=1))
        xin = ctx.enter_context(tc.tile_pool(name="xin", bufs=4))
        xtp = ctx.enter_context(tc.tile_pool(name="xtp", bufs=2, space="PSUM"))
        xts = ctx.enter_context(tc.tile_pool(name="xts", bufs=2))
        scp = ctx.enter_context(tc.tile_pool(name="scp", bufs=2, space="PSUM"))
        accp = ctx.enter_context(tc.tile_pool(name="accp", bufs=1, space="PSUM"))
        work = ctx.enter_context(tc.tile_pool(name="work", bufs=6))
        qsb = ctx.enter_context(tc.tile_pool(name="qsb", bufs=4))
        if GATHER_MODE == "pe":
            ohtp = ctx.enter_context(tc.tile_pool(name="ohtp", bufs=1,
                                                  space="PSUM"))
            qps = ctx.enter_context(tc.tile_pool(name="qps", bufs=2,
                                                 space="PSUM"))

        # ---- constants ----
        id128 = consts.tile([P, P], F32)
        make_identity(nc, id128[:])
        id128_bf = consts.tile([P, P], BF16)
        if GATHER_MODE == "pe":
            make_identity(nc, id128_bf[:])
        ones_col = consts.tile([P, 1], F32)
        nc.vector.memset(ones_col[:], 1.0)
        viota = consts.tile([P, K], F32)
        nc.gpsimd.iota(viota[:], pattern=[[1, K]], base=0, channel_multiplier=0,
                       allow_small_or_imprecise_dtypes=True)

        cb_sb = consts.tile([K, D], F32)
        nc.sync.dma_start(cb_sb[:], cb[:])
        # negc2[k] = -sum_d c[k,d]^2, as a broadcast row tile [P, K]
        cb_dump = consts.tile([K, D], F32)
        c2col = consts.tile([K, 1], F32)
        nc.scalar.activation(cb_dump[:], cb_sb[:], AF.Square,
                             accum_out=c2col[:])
        # c2 row replicated across partitions: PE transpose [K,1]->[1,K],
        # then ones-matmul broadcast to [P, K] is overkill — use matmul:
        # c2row_bc[p, k] = sum_j ones[j, p] * c2T[j, k], j = 1..1
        c2t_ps = xtp.tile([P, K], F32, tag="xt_ps")
        nc.tensor.transpose(c2t_ps[0:1, 0:K], c2col[:], id128[0:K, 0:K])
        c2row = consts.tile([1, K], F32)
        nc.scalar.copy(c2row[:], c2t_ps[0:1, 0:K])
        ones_row = consts.tile([1, P], F32)
        nc.vector.memset(ones_row[:], 1.0)
        c2bc_ps = xtp.tile([P, K], F32, tag="xt_ps")
        nc.tensor.matmul(c2bc_ps[:], ones_row[:], c2row[:])
        c2bc = consts.tile([P, K], F32)
        nc.scalar.copy(c2bc[:], c2bc_ps[:])

        # cT chunks: ct_sb[p, j, k] = codebook[k, j*128+p]
        ct_ps = xtp.tile([P, NCHUNK * K], F32, tag="xt_ps")
        for j in range(NCHUNK):
            nc.tensor.transpose(
                ct_ps[:, j * K:(j + 1) * K],
                cb_sb[:, j * P:(j + 1) * P],
                id128[0:K, 0:K],
            )
        ct_sb = consts.tile([P, NCHUNK, K], F32)
        nc.scalar.copy(ct_sb[:].rearrange("p j k -> p (j k)"), ct_ps[:])

        # bf16 codebook split for the gather matmul: cb = sum_i split_i
        cb_splits = []
        if GATHER_MODE == "pe":
            resid = cb_sb
            for i in range(QMM_SPLIT):
                h = consts.tile([K, D], BF16, tag=f"cbs{i}")
                nc.vector.tensor_copy(h[:], resid[:])
                if i + 1 < QMM_SPLIT:
                    r = consts.tile([K, D], F32, tag=f"cbr{i}")
                    nc.vector.tensor_tensor(out=r[:], in0=resid[:], in1=h[:],
                                            op=OP.subtract)
                    resid = r
                cb_splits.append(h)

        # persistent accumulator: [1, c, 0:K] counts, [1, c, K] maxsum
        acc_ps = accp.tile([1, tpg, K + 1], F32, tag="counts")

        for g in range(ng):
            xg = xin.tile([P, tpg, D], F32)
            nc.sync.dma_start(xg[:], xs_v[g])

            xt_g = work.tile([P, NCHUNK, group], F32, tag="xt_g")
            x2g = work.tile([P, tpg], F32, tag="x2g")
            # comb[:, c, 0:K] = onehot(tile c), comb[:, c, K] = max S(tile c)
            comb = work.tile([P, tpg, K + 1], F32, tag="comb")
            if GATHER_MODE == "pe":
                onehot_bf = work.tile([P, tpg, K], BF16, tag="onehot_bf")
            else:
                idxg = work.tile([P, tpg], I32, tag="idxg")

            for c in range(tpg):
                xt_ps = xtp.tile([P, D], F32, tag="xt_ps")
                for j in range(NCHUNK):
                    nc.tensor.transpose(
                        xt_ps[:, j * P:(j + 1) * P],
                        xg[:, c, j * P:(j + 1) * P],
                        id128[:],
                    )
                src = xt_ps[:].rearrange("p (j t) -> p j t", j=NCHUNK)
                nc.vector.tensor_copy(xt_g[:, :, c * P:(c + 1) * P], src)
                # x2 per token: square with accumulate
                if X2_ENGINE == "act":
                    sq_dump = work.tile([P, D], BF16, tag="sqdump")
                    nc.scalar.activation(
                        sq_dump[:], xg[:, c, :], AF.Square,
                        accum_out=x2g[:, c:c + 1],
                    )
                else:
                    # offload to the otherwise-idle GPSIMD engine
                    sq_dump = work.tile([P, D], F32, tag="sqdump")
                    nc.gpsimd.tensor_tensor(
                        out=sq_dump[:], in0=xg[:, c, :], in1=xg[:, c, :],
                        op=OP.mult)
                    acc_dump = work.tile([P, D], F32, tag="accdump")
                    nc.gpsimd.tensor_scalar(
                        out=acc_dump[:], in0=sq_dump[:], scalar1=1.0,
                        scalar2=None, op0=OP.mult, op1=OP.add,
                        accum_out=x2g[:, c:c + 1])

            for c in range(tpg):
                # dists (negated): S = 2*xc - x2 - c2, in [tok, K] layout
                sc_ps = scp.tile([P, K], F32, tag="sc")
                for j in range(NCHUNK):
                    nc.tensor.matmul(
                        sc_ps[:], xt_g[:, j, c * P:(c + 1) * P],
                        ct_sb[:, j, :],
                        start=(j == 0), stop=(j == NCHUNK - 1),
                    )
                s1 = work.tile([P, K], F32, tag="s1")
                nc.vector.tensor_scalar(
                    out=s1[:], in0=sc_ps[:], scalar1=2.0,
                    scalar2=x2g[:, c:c + 1], op0=OP.mult, op1=OP.subtract,
                )
                s_sb = work.tile([P, K], F32, tag="s")
                nc.vector.tensor_tensor(
                    out=s_sb[:], in0=s1[:], in1=c2bc[:], op=OP.subtract)
                vmax8 = work.tile([P, 8], F32, tag="vmax8")
                nc.vector.max(vmax8[:], s_sb[:])
                vidx = work.tile([P, 8], U32, tag="vidx")
                nc.vector.max_index(vidx[:], vmax8[:], s_sb[:])
                nc.vector.tensor_copy(comb[:, c, K:K + 1], vmax8[:, 0:1])
                idxf = work.tile([P, 1], F32, tag="idxf")
                nc.vector.tensor_copy(idxf[:], vidx[:, 0:1])
                if GATHER_MODE == "dma":
                    nc.vector.tensor_copy(idxg[:, c:c + 1], vidx[:, 0:1])
                nc.vector.tensor_scalar(
                    out=comb[:, c, 0:K], in0=viota[:], scalar1=idxf[:],
                    scalar2=None, op0=OP.is_equal,
                )

            # batched accumulator for counts and loss (one chain, one bank)
            first, last = (g == 0), (g == ng - 1)
            nc.tensor.matmul(
                acc_ps[:].rearrange("a c k -> a (c k)"), ones_col[:],
                comb[:].rearrange("p c k -> p (c k)"),
                start=first, stop=last, skip_group_check=True,
            )

            q_g = qsb.tile([P, tpg, D], F32)
            if GATHER_MODE == "pe":
                # one batched f32->bf16 copy of the group's onehots
                nc.vector.tensor_copy(onehot_bf[:], comb[:, :, 0:K])
                oht_ps = ohtp.tile([K, tpg, P], BF16, tag="oht")
                for c in range(tpg):
                    nc.tensor.transpose(
                        oht_ps[:, c, :], onehot_bf[:, c, :], id128_bf[:])
                oht_sb = work.tile([K, tpg, P], BF16, tag="oht_sb")
                nc.vector.tensor_copy(oht_sb[:], oht_ps[:])
                for c in range(tpg):
                    q_ps = qps.tile([P, D], F32, tag="q_ps")
                    for i, h in enumerate(cb_splits):
                        nc.tensor.matmul(
                            q_ps[:], oht_sb[:, c, :], h[:],
                            start=(i == 0), stop=(i == len(cb_splits) - 1),
                        )
                    # balance PSUM->SBUF copy load between ACT and DVE
                    if c == 0 and g % 2 == 0:
                        nc.vector.tensor_copy(q_g[:, c, :], q_ps[:])
                    else:
                        nc.scalar.copy(q_g[:, c, :], q_ps[:])
            else:
                for c in range(tpg):
                    nc.gpsimd.indirect_dma_start(
                        out=q_g[:, c, :], out_offset=None, in_=cb[:],
                        in_offset=IndirectOffsetOnAxis(
                            ap=idxg[:, c:c + 1], axis=0),
                    )
            nc.sync.dma_start(q_v[g], q_g[:])

        # final: fold per-subtile accumulators and ship stats
        stats_sb = consts.tile([1, K + tpg], F32)
        # counts: acc_ps [1, tpg, K+1] -> sum over c (view k-major, reduce X)
        nc.vector.tensor_reduce(
            stats_sb[0:1, 0:K],
            acc_ps[:, :, 0:K].rearrange("a c k -> a k c"),
            axis=AX.X, op=OP.add)
        nc.scalar.copy(stats_sb[0:1, K:K + tpg], acc_ps[:, :, K])
        nc.sync.dma_start(stats_out[:], stats_sb[:])

    return nc


_NC_CACHE = {}


def _get_nc(tok: int):
    if tok not in _NC_CACHE:
        nc = build_nc(tok)
        nc.finalize()
        _NC_CACHE[tok] = nc
    return _NC_CACHE[tok]


def run_cores(x: np.ndarray, codebook: np.ndarray, trace: bool = False):
    """x: [B, T, D] float32. Returns BassKernelResults."""
    b = x.shape[0]
    tok = x.shape[1]
    nc = _get_nc(tok)
    in_maps = [
        {"xs": np.ascontiguousarray(x[i]), "cb": np.ascontiguousarray(codebook)}
        for i in range(b)
    ]
    res = run_bass_kernel_spmd(nc, in_maps, list(range(b)), trace=trace)
    return res


def kernel(x: np.ndarray, codebook: np.ndarray):
    x = np.asarray(x, dtype=np.float32)
    codebook = np.asarray(codebook, dtype=np.float32)
    b, t, d = x.shape
    res = run_cores(x, codebook)
    outs = res.results
    quantized = np.stack([outs[i]["q"] for i in range(b)], axis=0).reshape(b, t, d)
    stats = np.stack([outs[i]["stats"][0] for i in range(b)], axis=0)
    counts = np.round(stats[:, :K].sum(axis=0)).astype(np.int32)
    maxsum = stats[:, K:].astype(np.float64).sum()
    vq_loss = np.float32((1.0 + COMMITMENT_COST) * (-maxsum) / (b * t * d))
    return quantized, vq_loss, counts


# revision 41
# speedup vs baseline: 1.0007x; 1.0007x over previous
"""VQ codebook (nn_ConceptGraph) Trainium2 kernel.

Data-parallel over batch B=8: one NeuronCore per batch row (8192 tokens each).
Per core:
  x [8192, 512] f32, codebook [32, 512] f32 (replicated).
  S[t, k] = 2*x.c_k - |x_t|^2 - |c_k|^2  (= -squared_dist)
  idx = argmax_k S  (== argmin dist, first-index tie-break via max8/max_index)
  quantized[t] = codebook[idx[t]]
  counts += onehot(idx), maxsum += max_k S (ones-matmul PSUM accumulation)
Host combines: counts = sum over cores; vq_loss = 1.25 * (-sum maxsum) / (B*T*D).

Pipeline per 512-token group (16 groups/core):
  DMA x -> PE transpose (xT) -> ACT copy to SBUF -> PE matmul vs cT
  (dists in [tok, 32] PSUM) -> DVE argmax chain -> PE onehot-matmul gather
  (bf16-split codebook for full-rate PE) -> DVE/ACT copy -> DMA out.
"""

import os
import sys

import numpy as np

sys.path.insert(0, "/opt/trn_rl_repo")

import concourse.bacc as bacc
import concourse.bass as bass
import concourse.mybir as mybir
import concourse.tile as tile
from concourse.bass import IndirectOffsetOnAxis
from concourse.bass_utils import run_bass_kernel_spmd
from concourse.masks import make_identity
from contextlib import ExitStack

F32 = mybir.dt.float32
BF16 = mybir.dt.bfloat16
I32 = mybir.dt.int32
U32 = mybir.dt.uint32
AF = mybir.ActivationFunctionType
OP = mybir.AluOpType
AX = mybir.AxisListType

P = 128          # partitions
D = 512          # model dim
K = 32           # codebook entries
B = 8            # batch (== n cores)
T = 8192         # tokens per core
NCHUNK = D // P  # 4 contraction chunks

COMMITMENT_COST = 0.25

# tensor_tensor_reduce crashes this runtime's devices — never use it.
GATHER_MODE = os.environ.get("VQ_GATHER", "pe")   # "pe" | "dma"
QMM_SPLIT = int(os.environ.get("VQ_QSPLIT", "3"))  # bf16 split terms for q-matmul
X2_ENGINE = os.environ.get("VQ_X2", "act")         # "act" | "pool"


def build_nc(tok: int = T, group: int = int(os.environ.get("VQ_GROUP", "512"))):
    ng = tok // group        # number of groups
    tpg = group // P         # tiles (of 128 tokens) per group

    nc = bacc.Bacc("TRN2", target_bir_lowering=False, debug=False)
    xs = nc.declare_dram_parameter("xs", [tok, D], F32, isOutput=False)
    cb = nc.declare_dram_parameter("cb", [K, D], F32, isOutput=False)
    q_out = nc.declare_dram_parameter("q", [tok, D], F32, isOutput=True)
    stats_out = nc.declare_dram_parameter("stats", [1, K + tpg], F32,
                                          isOutput=True)

    # DRAM views: token index = g*group + c*128 + p
    xs_v = xs[:].rearrange("(g c p) d -> g p c d", p=P, c=tpg)
    q_v = q_out[:].rearrange("(g c p) d -> g p c d", p=P, c=tpg)

    with ExitStack() as ctx:
        tc = ctx.enter_context(tile.TileContext(nc))
        consts = ctx.enter_context(tc.tile_pool(name="consts", bufs=1))
        xin = ctx.enter_context(tc.tile_pool(name="xin", bufs=4 if group <= 512 else 2))
        xtp = ctx.enter_context(tc.tile_pool(name="xtp", bufs=2, space="PSUM"))
        scp = ctx.enter_context(tc.tile_pool(name="scp", bufs=2, space="PSUM"))
        accp = ctx.enter_context(tc.tile_pool(name="accp", bufs=1, space="PSUM"))
        work = ctx.enter_context(tc.tile_pool(name="work", bufs=8))
        bigw = ctx.enter_context(tc.tile_pool(name="bigw", bufs=4 if group <= 512 else 3))
        qsb = ctx.enter_context(tc.tile_pool(name="qsb", bufs=4 if group <= 512 else 2))
        if GATHER_MODE == "pe":
            ohtp = ctx.enter_context(tc.tile_pool(name="ohtp", bufs=1,
                                                  space="PSUM"))
            qps = ctx.enter_context(tc.tile_pool(name="qps", bufs=2,
                                                 space="PSUM"))

        # ---- constants ----
        id128 = consts.tile([P, P], F32)
        make_identity(nc, id128[:])
        id128_bf = consts.tile([P, P], BF16)
        if GATHER_MODE == "pe":
            make_identity(nc, id128_bf[:])
        ones_col = consts.tile([P, 1], F32)
        nc.vector.memset(ones_col[:], 1.0)
        viota = consts.tile([P, K], F32)
        nc.gpsimd.iota(viota[:], pattern=[[1, K]], base=0, channel_multiplier=0,
                       allow_small_or_imprecise_dtypes=True)

        cb_sb = consts.tile([K, D], F32)
        nc.sync.dma_start(cb_sb[:], cb[:])
        # negc2[k] = -sum_d c[k,d]^2, as a broadcast row tile [P, K]
        cb_dump = consts.tile([K, D], F32)
        c2col = consts.tile([K, 1], F32)
        nc.scalar.activation(cb_dump[:], cb_sb[:], AF.Square,
                             accum_out=c2col[:])
        # c2 row replicated across partitions: PE transpose [K,1]->[1,K],
        # then ones-matmul broadcast to [P, K] is overkill — use matmul:
        # c2row_bc[p, k] = sum_j ones[j, p] * c2T[j, k], j = 1..1
        c2t_ps = xtp.tile([P, K], F32, tag="xt_ps")
        nc.tensor.transpose(c2t_ps[0:1, 0:K], c2col[:], id128[0:K, 0:K])
        c2row = consts.tile([1, K], F32)
        nc.scalar.copy(c2row[:], c2t_ps[0:1, 0:K])
        ones_row = consts.tile([1, P], F32)
        nc.vector.memset(ones_row[:], 1.0)
        c2bc_ps = xtp.tile([P, K], F32, tag="xt_ps")
        nc.tensor.matmul(c2bc_ps[:], ones_row[:], c2row[:])
        c2bc = consts.tile([P, K], F32)
        nc.scalar.copy(c2bc[:], c2bc_ps[:])

        # cT chunks: ct_sb[p, j, k] = codebook[k, j*128+p]
        ct_ps = xtp.tile([P, NCHUNK * K], F32, tag="xt_ps")
        for j in range(NCHUNK):
            nc.tensor.transpose(
                ct_ps[:, j * K:(j + 1) * K],
                cb_sb[:, j * P:(j + 1) * P],
                id128[0:K, 0:K],
            )
        ct_sb = consts.tile([P, NCHUNK, K], F32)
        nc.scalar.copy(ct_sb[:].rearrange("p j k -> p (j k)"), ct_ps[:])

        # bf16 codebook split for the gather matmul: cb = sum_i split_i
        cb_splits = []
        if GATHER_MODE == "pe":
            resid = cb_sb
            for i in range(QMM_SPLIT):
                h = consts.tile([K, D], BF16, tag=f"cbs{i}")
                nc.vector.tensor_copy(h[:], resid[:])
                if i + 1 < QMM_SPLIT:
                    r = consts.tile([K, D], F32, tag=f"cbr{i}")
                    nc.vector.tensor_tensor(out=r[:], in0=resid[:], in1=h[:],
                                            op=OP.subtract)
                    resid = r
                cb_splits.append(h)

        # persistent accumulator: [1, c, 0:K] counts, [1, c, K] maxsum
        acc_ps = accp.tile([1, tpg, K + 1], F32, tag="counts")

        for g in range(ng):
            xg = xin.tile([P, tpg, D], F32)
            nc.sync.dma_start(xg[:], xs_v[g])

            xt_g = bigw.tile([P, NCHUNK, group], F32, tag="xt_g")
            x2g = work.tile([P, tpg], F32, tag="x2g")
            # comb[:, c, 0:K] = onehot(tile c), comb[:, c, K] = max S(tile c)
            comb = work.tile([P, tpg, K + 1], F32, tag="comb")
            if GATHER_MODE == "pe":
                onehot_bf = work.tile([P, tpg, K], BF16, tag="onehot_bf")
            else:
                idxg = work.tile([P, tpg], I32, tag="idxg")

            for c in range(tpg):
                xt_ps = xtp.tile([P, D], F32, tag="xt_ps")
                for j in range(NCHUNK):
                    nc.tensor.transpose(
                        xt_ps[:, j * P:(j + 1) * P],
                        xg[:, c, j * P:(j + 1) * P],
                        id128[:],
                    )
                src = xt_ps[:].rearrange("p (j t) -> p j t", j=NCHUNK)
                nc.vector.tensor_copy(xt_g[:, :, c * P:(c + 1) * P], src)
                # x2 per token: square with accumulate
                if X2_ENGINE == "act":
                    sq_dump = bigw.tile([P, D], BF16, tag="sqdump")
                    nc.scalar.activation(
                        sq_dump[:], xg[:, c, :], AF.Square,
                        accum_out=x2g[:, c:c + 1],
                    )
                else:
                    # offload to the otherwise-idle GPSIMD engine
                    sq_dump = bigw.tile([P, D], F32, tag="sqdump")
                    nc.gpsimd.tensor_tensor(
                        out=sq_dump[:], in0=xg[:, c, :], in1=xg[:, c, :],
                        op=OP.mult)
                    acc_dump = bigw.tile([P, D], F32, tag="accdump")
                    nc.gpsimd.tensor_scalar(
                        out=acc_dump[:], in0=sq_dump[:], scalar1=1.0,
                        scalar2=None, op0=OP.mult, op1=OP.add,
                        accum_out=x2g[:, c:c + 1])

            for c in range(tpg):
                # dists (negated): S = 2*xc - x2 - c2, in [tok, K] layout
                sc_ps = scp.tile([P, K], F32, tag="sc")
                for j in range(NCHUNK):
                    nc.tensor.matmul(
                        sc_ps[:], xt_g[:, j, c * P:(c + 1) * P],
                        ct_sb[:, j, :],
                        start=(j == 0), stop=(j == NCHUNK - 1),
                    )
                s1 = work.tile([P, K], F32, tag="s1")
                nc.vector.tensor_scalar(
                    out=s1[:], in0=sc_ps[:], scalar1=2.0,
                    scalar2=x2g[:, c:c + 1], op0=OP.mult, op1=OP.subtract,
                )
                s_sb = work.tile([P, K], F32, tag="s")
                nc.vector.tensor_tensor(
                    out=s_sb[:], in0=s1[:], in1=c2bc[:], op=OP.subtract)
                vmax8 = work.tile([P, 8], F32, tag="vmax8")
                nc.vector.max(vmax8[:], s_sb[:])
                vidx = work.tile([P, 8], U32, tag="vidx")
                nc.vector.max_index(vidx[:], vmax8[:], s_sb[:])
                nc.vector.tensor_copy(comb[:, c, K:K + 1], vmax8[:, 0:1])
                idxf = work.tile([P, 1], F32, tag="idxf")
                nc.vector.tensor_copy(idxf[:], vidx[:, 0:1])
                if GATHER_MODE == "dma":
                    nc.vector.tensor_copy(idxg[:, c:c + 1], vidx[:, 0:1])
                nc.vector.tensor_scalar(
                    out=comb[:, c, 0:K], in0=viota[:], scalar1=idxf[:],
                    scalar2=None, op0=OP.is_equal,
                )

            # batched accumulator for counts and loss (one chain, one bank)
            first, last = (g == 0), (g == ng - 1)
            nc.tensor.matmul(
                acc_ps[:].rearrange("a c k -> a (c k)"), ones_col[:],
                comb[:].rearrange("p c k -> p (c k)"),
                start=first, stop=last, skip_group_check=True,
            )

            q_g = qsb.tile([P, tpg, D], F32)
            if GATHER_MODE == "pe":
                # one batched f32->bf16 copy of the group's onehots
                nc.vector.tensor_copy(onehot_bf[:], comb[:, :, 0:K])
                oht_ps = ohtp.tile([K, tpg, P], BF16, tag="oht")
                for c in range(tpg):
                    nc.tensor.transpose(
                        oht_ps[:, c, :], onehot_bf[:, c, :], id128_bf[:])
                oht_sb = work.tile([K, tpg, P], BF16, tag="oht_sb")
                nc.vector.tensor_copy(oht_sb[:], oht_ps[:])
                for c in range(tpg):
                    q_ps = qps.tile([P, D], F32, tag="q_ps")
                    for i, h in enumerate(cb_splits):
                        nc.tensor.matmul(
                            q_ps[:], oht_sb[:, c, :], h[:],
                            start=(i == 0), stop=(i == len(cb_splits) - 1),
                        )
                    # balance PSUM->SBUF copy load between ACT and DVE
                    if c == 0 and g % 2 == 0:
                        nc.vector.tensor_copy(q_g[:, c, :], q_ps[:])
                    else:
                        nc.scalar.copy(q_g[:, c, :], q_ps[:])
            else:
                for c in range(tpg):
                    nc.gpsimd.indirect_dma_start(
                        out=q_g[:, c, :], out_offset=None, in_=cb[:],
                        in_offset=IndirectOffsetOnAxis(
                            ap=idxg[:, c:c + 1], axis=0),
                    )
            nc.sync.dma_start(q_v[g], q_g[:])

        # final: fold per-subtile accumulators and ship stats
        stats_sb = consts.tile([1, K + tpg], F32)
        # counts: acc_ps [1, tpg, K+1] -> sum over c (view k-major, reduce X)
        nc.vector.tensor_reduce(
            stats_sb[0:1, 0:K],
            acc_ps[:, :, 0:K].rearrange("a c k -> a k c"),
            axis=AX.X, op=OP.add)
        nc.scalar.copy(stats_sb[0:1, K:K + tpg], acc_ps[:, :, K])
        nc.sync.dma_start(stats_out[:], stats_sb[:])

    return nc


_NC_CACHE = {}


def _get_nc(tok: int):
    if tok not in _NC_CACHE:
        nc = build_nc(tok)
        nc.finalize()
        _NC_CACHE[tok] = nc
    return _NC_CACHE[tok]


def run_cores(x: np.ndarray, codebook: np.ndarray, trace: bool = False):
    """x: [B, T, D] float32. Returns BassKernelResults."""
    b = x.shape[0]
    tok = x.shape[1]
    nc = _get_nc(tok)
    in_maps = [
        {"xs": np.ascontiguousarray(x[i]), "cb": np.ascontiguousarray(codebook)}
        for i in range(b)
    ]
    res = run_bass_kernel_spmd(nc, in_maps, list(range(b)), trace=trace)
    return res


def kernel(x: np.ndarray, codebook: np.ndarray):
    x = np.asarray(x, dtype=np.float32)
    codebook = np.asarray(codebook, dtype=np.float32)
    b, t, d = x.shape
    res = run_cores(x, codebook)
    outs = res.results
    quantized = np.stack([outs[i]["q"] for i in range(b)], axis=0).reshape(b, t, d)
    stats = np.stack([outs[i]["stats"][0] for i in range(b)], axis=0)
    counts = np.round(stats[:, :K].sum(axis=0)).astype(np.int32)
    maxsum = stats[:, K:].astype(np.float64).sum()
    vq_loss = np.float32((1.0 + COMMITMENT_COST) * (-maxsum) / (b * t * d))
    return quantized, vq_loss, counts


# revision 46
# speedup vs baseline: 1.0271x; 1.0264x over previous
"""VQ codebook (nn_ConceptGraph) Trainium2 kernel.

Data-parallel over batch B=8: one NeuronCore per batch row (8192 tokens each).
Per core:
  x [8192, 512] f32, codebook [32, 512] f32 (replicated).
  S[t, k] = 2*x.c_k - |x_t|^2 - |c_k|^2  (= -squared_dist)
  idx = argmax_k S  (== argmin dist, first-index tie-break via max8/max_index)
  quantized[t] = codebook[idx[t]]
  counts += onehot(idx), maxsum += max_k S (ones-matmul PSUM accumulation)
Host combines: counts = sum over cores; vq_loss = 1.25 * (-sum maxsum) / (B*T*D).

Pipeline per 512-token group (16 groups/core):
  DMA x -> PE transpose (xT) -> ACT copy to SBUF -> PE matmul vs cT
  (dists in [tok, 32] PSUM) -> DVE argmax chain -> PE onehot-matmul gather
  (bf16-split codebook for full-rate PE) -> DVE/ACT copy -> DMA out.
"""

import os
import sys

import numpy as np

sys.path.insert(0, "/opt/trn_rl_repo")

import concourse.bacc as bacc
import concourse.bass as bass
import concourse.mybir as mybir
import concourse.tile as tile
from concourse.bass import IndirectOffsetOnAxis
from concourse.bass_utils import run_bass_kernel_spmd
from concourse.masks import make_identity
from contextlib import ExitStack

F32 = mybir.dt.float32
BF16 = mybir.dt.bfloat16
I32 = mybir.dt.int32
U32 = mybir.dt.uint32
AF = mybir.ActivationFunctionType
OP = mybir.AluOpType
AX = mybir.AxisListType

P = 128          # partitions
D = 512          # model dim
K = 32           # codebook entries
B = 8            # batch (== n cores)
T = 8192         # tokens per core
NCHUNK = D // P  # 4 contraction chunks

COMMITMENT_COST = 0.25

# tensor_tensor_reduce crashes this runtime's devices — never use it.
GATHER_MODE = os.environ.get("VQ_GATHER", "pe")   # "pe" | "dma"
QMM_SPLIT = int(os.environ.get("VQ_QSPLIT", "3"))  # bf16 split terms for q-matmul
X2_ENGINE = os.environ.get("VQ_X2", "act")         # "act" | "pool"


def build_nc(tok: int = T, group: int = int(os.environ.get("VQ_GROUP", "512"))):
    ng = tok // group        # number of groups
    tpg = group // P         # tiles (of 128 tokens) per group

    nc = bacc.Bacc("TRN2", target_bir_lowering=False, debug=False)
    xs = nc.declare_dram_parameter("xs", [tok, D], F32, isOutput=False)
    cb = nc.declare_dram_parameter("cb", [K, D], F32, isOutput=False)
    q_out = nc.declare_dram_parameter("q", [tok, D], F32, isOutput=True)
    stats_out = nc.declare_dram_parameter("stats", [1, K + tpg], F32,
                                          isOutput=True)

    # DRAM views: token index = g*group + c*128 + p
    xs_v = xs[:].rearrange("(g c p) d -> g p c d", p=P, c=tpg)
    q_v = q_out[:].rearrange("(g c p) d -> g p c d", p=P, c=tpg)

    with ExitStack() as ctx:
        tc = ctx.enter_context(tile.TileContext(nc))
        consts = ctx.enter_context(tc.tile_pool(name="consts", bufs=1))
        xin = ctx.enter_context(tc.tile_pool(name="xin", bufs=4 if group <= 512 else 2))
        xtp = ctx.enter_context(tc.tile_pool(name="xtp", bufs=2, space="PSUM"))
        scp = ctx.enter_context(tc.tile_pool(name="scp", bufs=2, space="PSUM"))
        accp = ctx.enter_context(tc.tile_pool(name="accp", bufs=1, space="PSUM"))
        work = ctx.enter_context(tc.tile_pool(name="work", bufs=8))
        bigw = ctx.enter_context(tc.tile_pool(name="bigw", bufs=4 if group <= 512 else 3))
        qsb = ctx.enter_context(tc.tile_pool(name="qsb", bufs=4 if group <= 512 else 2))
        if GATHER_MODE == "pe":
            ohtp = ctx.enter_context(tc.tile_pool(name="ohtp", bufs=1,
                                                  space="PSUM"))
            qps = ctx.enter_context(tc.tile_pool(name="qps", bufs=2,
                                                 space="PSUM"))

        # ---- constants ----
        id128 = consts.tile([P, P], F32)
        make_identity(nc, id128[:])
        id128_bf = consts.tile([P, P], BF16)
        if GATHER_MODE == "pe":
            make_identity(nc, id128_bf[:])
        ones_col = consts.tile([P, 1], F32)
        nc.vector.memset(ones_col[:], 1.0)
        viota = consts.tile([P, K], F32)
        nc.gpsimd.iota(viota[:], pattern=[[1, K]], base=0, channel_multiplier=0,
                       allow_small_or_imprecise_dtypes=True)

        cb_sb = consts.tile([K, D], F32)
        nc.sync.dma_start(cb_sb[:], cb[:])
        # negc2[k] = -sum_d c[k,d]^2, as a broadcast row tile [P, K]
        cb_dump = consts.tile([K, D], F32)
        c2col = consts.tile([K, 1], F32)
        nc.scalar.activation(cb_dump[:], cb_sb[:], AF.Square,
                             accum_out=c2col[:])
        # c2 row replicated across partitions: PE transpose [K,1]->[1,K],
        # then ones-matmul broadcast to [P, K] is overkill — use matmul:
        # c2row_bc[p, k] = sum_j ones[j, p] * c2T[j, k], j = 1..1
        c2t_ps = xtp.tile([P, K], F32, tag="xt_ps")
        nc.tensor.transpose(c2t_ps[0:1, 0:K], c2col[:], id128[0:K, 0:K])
        c2row = consts.tile([1, K], F32)
        nc.scalar.copy(c2row[:], c2t_ps[0:1, 0:K])
        ones_row = consts.tile([1, P], F32)
        nc.vector.memset(ones_row[:], 1.0)
        c2bc_ps = xtp.tile([P, K], F32, tag="xt_ps")
        nc.tensor.matmul(c2bc_ps[:], ones_row[:], c2row[:])
        c2bc = consts.tile([P, K], F32)
        nc.scalar.copy(c2bc[:], c2bc_ps[:])

        # cT chunks: ct_sb[p, j, k] = codebook[k, j*128+p]
        ct_ps = xtp.tile([P, NCHUNK * K], F32, tag="xt_ps")
        for j in range(NCHUNK):
            nc.tensor.transpose(
                ct_ps[:, j * K:(j + 1) * K],
                cb_sb[:, j * P:(j + 1) * P],
                id128[0:K, 0:K],
            )
        ct_sb = consts.tile([P, NCHUNK, K], F32)
        nc.scalar.copy(ct_sb[:].rearrange("p j k -> p (j k)"), ct_ps[:])

        # bf16 codebook split for the gather matmul: cb = sum_i split_i
        cb_splits = []
        if GATHER_MODE == "pe":
            resid = cb_sb
            for i in range(QMM_SPLIT):
                h = consts.tile([K, D], BF16, tag=f"cbs{i}")
                nc.vector.tensor_copy(h[:], resid[:])
                if i + 1 < QMM_SPLIT:
                    r = consts.tile([K, D], F32, tag=f"cbr{i}")
                    nc.vector.tensor_tensor(out=r[:], in0=resid[:], in1=h[:],
                                            op=OP.subtract)
                    resid = r
                cb_splits.append(h)

        # persistent accumulator: [1, c, 0:K] counts, [1, c, K] maxsum
        acc_ps = accp.tile([1, tpg, K + 1], F32, tag="counts")

        for g in range(ng):
            xg = xin.tile([P, tpg, D], F32)
            nc.sync.dma_start(xg[:], xs_v[g])

            xt_g = bigw.tile([P, NCHUNK, group], F32, tag="xt_g")
            x2g = work.tile([P, tpg], F32, tag="x2g")
            # comb[:, c, 0:K] = onehot(tile c), comb[:, c, K] = max S(tile c)
            comb = work.tile([P, tpg, K + 1], F32, tag="comb")
            if GATHER_MODE == "pe":
                onehot_bf = work.tile([P, tpg, K], BF16, tag="onehot_bf")
            else:
                idxg = work.tile([P, tpg], I32, tag="idxg")

            for c in range(tpg):
                xt_ps = xtp.tile([P, D], F32, tag="xt_ps")
                for j in range(NCHUNK):
                    nc.tensor.transpose(
                        xt_ps[:, j * P:(j + 1) * P],
                        xg[:, c, j * P:(j + 1) * P],
                        id128[:],
                    )
                src = xt_ps[:].rearrange("p (j t) -> p j t", j=NCHUNK)
                nc.vector.tensor_copy(xt_g[:, :, c * P:(c + 1) * P], src)
                # x2 per token: square with accumulate
                if X2_ENGINE == "act":
                    sq_dump = bigw.tile([P, D], BF16, tag="sqdump")
                    nc.scalar.activation(
                        sq_dump[:], xg[:, c, :], AF.Square,
                        accum_out=x2g[:, c:c + 1],
                    )
                else:
                    # offload to the otherwise-idle GPSIMD engine
                    sq_dump = bigw.tile([P, D], F32, tag="sqdump")
                    nc.gpsimd.tensor_tensor(
                        out=sq_dump[:], in0=xg[:, c, :], in1=xg[:, c, :],
                        op=OP.mult)
                    acc_dump = bigw.tile([P, D], F32, tag="accdump")
                    nc.gpsimd.tensor_scalar(
                        out=acc_dump[:], in0=sq_dump[:], scalar1=1.0,
                        scalar2=None, op0=OP.mult, op1=OP.add,
                        accum_out=x2g[:, c:c + 1])

            for c in range(tpg):
                # dists (negated): S = 2*xc - x2 - c2, in [tok, K] layout
                sc_ps = scp.tile([P, K], F32, tag="sc")
                for j in range(NCHUNK):
                    nc.tensor.matmul(
                        sc_ps[:], xt_g[:, j, c * P:(c + 1) * P],
                        ct_sb[:, j, :],
                        start=(j == 0), stop=(j == NCHUNK - 1),
                    )
                s1 = work.tile([P, K], F32, tag="s1")
                nc.vector.tensor_scalar(
                    out=s1[:], in0=sc_ps[:], scalar1=2.0,
                    scalar2=x2g[:, c:c + 1], op0=OP.mult, op1=OP.subtract,
                )
                s_sb = work.tile([P, K], F32, tag="s")
                nc.vector.tensor_tensor(
                    out=s_sb[:], in0=s1[:], in1=c2bc[:], op=OP.subtract)
                vmax8 = work.tile([P, 8], F32, tag="vmax8")
                nc.vector.max(vmax8[:], s_sb[:])
                vidx = work.tile([P, 8], U32, tag="vidx")
                nc.vector.max_index(vidx[:], vmax8[:], s_sb[:])
                nc.vector.tensor_copy(comb[:, c, K:K + 1], vmax8[:, 0:1])
                idxf = work.tile([P, 1], F32, tag="idxf")
                nc.vector.tensor_copy(idxf[:], vidx[:, 0:1])
                if GATHER_MODE == "dma":
                    nc.vector.tensor_copy(idxg[:, c:c + 1], vidx[:, 0:1])
                nc.vector.tensor_scalar(
                    out=comb[:, c, 0:K], in0=viota[:], scalar1=idxf[:],
                    scalar2=None, op0=OP.is_equal,
                )

            # batched accumulator for counts and loss (one chain, one bank)
            first, last = (g == 0), (g == ng - 1)
            nc.tensor.matmul(
                acc_ps[:].rearrange("a c k -> a (c k)"), ones_col[:],
                comb[:].rearrange("p c k -> p (c k)"),
                start=first, stop=last, skip_group_check=True,
            )

            q_g = qsb.tile([P, tpg, D], F32)
            if GATHER_MODE == "pe":
                # one batched f32->bf16 copy of the group's onehots
                nc.vector.tensor_copy(onehot_bf[:], comb[:, :, 0:K])
                oht_ps = ohtp.tile([K, tpg, P], BF16, tag="oht")
                for c in range(tpg):
                    nc.tensor.transpose(
                        oht_ps[:, c, :], onehot_bf[:, c, :], id128_bf[:])
                oht_sb = work.tile([K, tpg, P], BF16, tag="oht_sb")
                nc.vector.tensor_copy(oht_sb[:], oht_ps[:])
                for c in range(tpg):
                    q_ps = qps.tile([P, D], F32, tag="q_ps")
                    for i, h in enumerate(cb_splits):
                        nc.tensor.matmul(
                            q_ps[:], oht_sb[:, c, :], h[:],
                            start=(i == 0), stop=(i == len(cb_splits) - 1),
                        )
                    # balance PSUM->SBUF copy load between ACT and DVE
                    if c == 0 and g % 2 == 0:
                        nc.vector.tensor_copy(q_g[:, c, :], q_ps[:])
                    else:
                        nc.scalar.copy(q_g[:, c, :], q_ps[:])
                    if c % 2 == 1:
                        nc.sync.dma_start(q_v[g][:, c - 1:c + 1, :],
                                          q_g[:, c - 1:c + 1, :])
            else:
                for c in range(tpg):
                    nc.gpsimd.indirect_dma_start(
                        out=q_g[:, c, :], out_offset=None, in_=cb[:],
                        in_offset=IndirectOffsetOnAxis(
                            ap=idxg[:, c:c + 1], axis=0),
                    )
            if GATHER_MODE != "pe":
                nc.sync.dma_start(q_v[g], q_g[:])

        # final: fold per-subtile accumulators and ship stats
        stats_sb = consts.tile([1, K + tpg], F32)
        # counts: acc_ps [1, tpg, K+1] -> sum over c (view k-major, reduce X)
        nc.vector.tensor_reduce(
            stats_sb[0:1, 0:K],
            acc_ps[:, :, 0:K].rearrange("a c k -> a k c"),
            axis=AX.X, op=OP.add)
        nc.scalar.copy(stats_sb[0:1, K:K + tpg], acc_ps[:, :, K])
        nc.sync.dma_start(stats_out[:], stats_sb[:])

    return nc


_NC_CACHE = {}


def _get_nc(tok: int):
    if tok not in _NC_CACHE:
        nc = build_nc(tok)
        nc.finalize()
        _NC_CACHE[tok] = nc
    return _NC_CACHE[tok]


def run_cores(x: np.ndarray, codebook: np.ndarray, trace: bool = False):
    """x: [B, T, D] float32. Returns BassKernelResults."""
    b = x.shape[0]
    tok = x.shape[1]
    nc = _get_nc(tok)
    in_maps = [
        {"xs": np.ascontiguousarray(x[i]), "cb": np.ascontiguousarray(codebook)}
        for i in range(b)
    ]
    res = run_bass_kernel_spmd(nc, in_maps, list(range(b)), trace=trace)
    return res


def kernel(x: np.ndarray, codebook: np.ndarray):
    x = np.asarray(x, dtype=np.float32)
    codebook = np.asarray(codebook, dtype=np.float32)
    b, t, d = x.shape
    res = run_cores(x, codebook)
    outs = res.results
    quantized = np.stack([outs[i]["q"] for i in range(b)], axis=0).reshape(b, t, d)
    stats = np.stack([outs[i]["stats"][0] for i in range(b)], axis=0)
    counts = np.round(stats[:, :K].sum(axis=0)).astype(np.int32)
    maxsum = stats[:, K:].astype(np.float64).sum()
    vq_loss = np.float32((1.0 + COMMITMENT_COST) * (-maxsum) / (b * t * d))
    return quantized, vq_loss, counts
